# revision 20
# baseline (speedup 1.0000x reference)
"""Tensor-parallel GQA attention (sigmoid-gated) for Trainium2, 8 NeuronCores.

Problem: B=2, S=2048, D=2048, H=32 q-heads, KV=8 kv-heads, HD=64 (GQA groups=4),
RoPE on q/k, full (non-causal) softmax, sigmoid(gate) output gating, out proj.

Sharding (tensor-parallel over heads): core c owns q-heads 4c..4c+3, kv-head c,
the matching 256 q-cols + 256 gate-cols of Wq, 64-col slices of Wk/Wv, and rows
256c:256c+256 of Wo. Each core computes a full [B*S, D] partial of the output
projection; the host sums the 8 partials.

Per-core pipeline (matmuls in float32r = full-rate reduced-precision fp32):
  A) projections psum[m,t] += W[d,m].T @ hsT[d,t] with per-dc weight DMAs so
     the first matmul starts as soon as one 128-row weight slice lands.
     q lands in head-PAIR layout qP[128, 2, T] (head 2m at partitions 0:64,
     head 2m+1 at 64:128); k is evicted to kv2lo (rows 0:64, zeros above) and
     mirrored into kv2hi (rows 64:128, zeros below) so each head's scores
     contract K=128 against a zero-padded k copy matching its partition half.
     RoPE is applied per 512-column chunk right after eviction (DVE + a few
     ops on the idle Pool/GPSIMD engine), fully overlapped with the next
     chunk's matmuls.  Gate columns get sigmoid at eviction and round-trip
     through DRAM (SBUF pressure).
  B) attention in scoresT orientation per 512-col i-segment, j in pairs:
     scoresT[j,i] = kv[hd,j].T @ qP[hd,i] for two j-tiles into one
     [128,1024] psum; probs via exp(s/8) on ACT (scale=0.125) for most
     pairs and a 1-op Schraudolph bit-trick exp on DVE for a few pairs
     (splits the activation load; softmax renormalization absorbs most of
     the approximation error).  attnT[hd,i] += v1[j, hd|1].T @ probs --
     v1's ones-column accumulates denominators in psum row 64 for free.
     Normalization is deferred one segment (runs under the next segment's
     matmuls): denom row -> SBUF, broadcast across partitions with a
     row-64-selector matmul, reciprocal_approx_fast, then gated muls
     (gate mul on the Pool engine).
  C) out[t,dout] += attnG[m,t].T @ Wo[m,dout] partials, interleaved per
     4-t-tile chunks between batch-1 heads so psum/eviction load spreads.
"""

import sys

sys.path.insert(0, "/opt/trn_rl_repo")

import numpy as np

import concourse.bass as bass  # noqa: F401
import concourse.mybir as mybir
import concourse.tile as tile
from concourse import bacc
from concourse.bass_utils import run_bass_kernel_spmd

F32 = mybir.dt.float32
F32R = mybir.dt.float32r
I32 = mybir.dt.int32
BF16 = mybir.dt.bfloat16
AF = mybir.ActivationFunctionType
ALU = mybir.AluOpType

P = 128
B, S, D = 2, 2048, 2048
T = B * S                  # 4096 token rows (batch folded)
H, KV, HD = 32, 8, 64
HH = HD // 2               # 32
NCORES = 8
NH = H // NCORES           # 4 q-heads per core
MQ = NH * HD               # 256 q-cols per core
DC = D // P                # 16 contraction chunks
TCH = 512                  # moving-dim chunk
NTCH = T // TCH            # 8
SJ = S // P                # 16 key chunks per batch
NSEG = S // TCH            # 4 i-segments per batch
NT = T // P                # 32 t-tiles
NP = SJ // 2               # 8 j-pairs per segment

# Schraudolph fast-exp (DVE): exp(s/8) ~= bitcast_i32_f32(s*AS + BS)
AS_CONST = float((1 << 23) * 1.4426950408889634 * 0.125)
BS_CONST = float(127 * (1 << 23) - 449200)
# which j-chunks (of 16 per i-half) compute probs on DVE instead of ACT
DVE_JCS = (3, 7, 11)


def build_nc(nreps=1):
    nc = bacc.Bacc("TRN2", target_bir_lowering=False, debug=False)

    hsT = nc.dram_tensor("hsT", [D, T], BF16, kind="ExternalInput")
    wqg = nc.dram_tensor("wqg", [D, 2 * MQ], BF16, kind="ExternalInput")
    wkv = nc.dram_tensor("wkv", [D, 2 * HD], BF16, kind="ExternalInput")
    wo = nc.dram_tensor("wo", [MQ, D], BF16, kind="ExternalInput")
    # rope tables, [128, S] with rows duplicated (row p holds entry p % 64)
    ck = nc.dram_tensor("ck", [P, S], F32, kind="ExternalInput")   # cos
    sk = nc.dram_tensor("sk", [P, S], F32, kind="ExternalInput")   # signed sin
    identd = nc.dram_tensor("ident", [HD, HD], F32R, kind="ExternalInput")
    oseld = nc.dram_tensor("osel", [P, P], F32R, kind="ExternalInput")  # row64=1
    protd = nc.dram_tensor("prot", [P, P], F32R, kind="ExternalInput")  # xor-32 perm
    onesd = nc.dram_tensor("ones", [P, B * SJ], BF16, kind="ExternalInput")
    out = nc.dram_tensor("out", [T, D], BF16, kind="ExternalOutput")

    gbuf = nc.dram_tensor("gbuf", [P, 2, T], BF16)  # sigmoid(gate), internal

    hsT3 = hsT.ap().rearrange("(o p) t -> p o t", p=P)   # [128, 16, 4096]
    wqg3 = wqg.ap().rearrange("(o p) m -> p o m", p=P)   # [128, 16, 512]
    wkv3 = wkv.ap().rearrange("(o p) m -> p o m", p=P)   # [128, 16, 128]
    wo3 = wo.ap().rearrange("(o p) n -> p o n", p=P)     # [128, 2, 2048]

    with tile.TileContext(nc) as tc:
        for _rep in range(nreps):
            with (
                tc.tile_pool(name="const", bufs=1) as const,
                tc.tile_pool(name="big", bufs=1) as big,
            ):
                ident_sb = const.tile([HD, HD], F32R)
                osel_sb = const.tile([P, P], F32R)  # row 64 ones, rest zero
                prot_sb = const.tile([P, P], F32R)  # xor-32 permutation
                nc.sync.dma_start(ident_sb[:], identd.ap())
                nc.sync.dma_start(osel_sb[:], oseld.ap())
                nc.sync.dma_start(prot_sb[:], protd.ap())

                # ---- persistent activations ----
                qP_sb = big.tile([P, 2, T], F32R)    # head pairs, roped in place
                kv2lo = big.tile([P, T], F32R)       # roped k rows 0:64, 0 above
                kv2hi = big.tile([P, T], F32R)       # 0 below, roped k rows 64:128
                v1_sb = big.tile([P, B * SJ, P], BF16)  # v | ones col | zeros

                # ---- stage A: projections + fused rope ----
                with (
                    nc.named_scope("stageA"),
                    tc.tile_pool(name="wpool", bufs=1) as wpool,
                    tc.tile_pool(name="tab", bufs=1) as tab,
                    tc.tile_pool(name="hst", bufs=3) as hst_pool,
                    tc.tile_pool(name="vst", bufs=8) as vst,
                    tc.tile_pool(name="evc", bufs=3) as evc,
                    tc.tile_pool(name="ps512", bufs=5, space="PSUM") as ps512,
                    tc.tile_pool(name="psrot", bufs=2, space="PSUM") as psrot,
                ):
                    ck_sb = tab.tile([P, S], F32)
                    sk_sb = tab.tile([P, S], F32)

                    wqg_sb = wpool.tile([P, DC, 2 * MQ], BF16)
                    wkv_sb = wpool.tile([P, DC, 2 * HD], BF16)
                    # zero pads: scores contract K=128 with a zeroed half so
                    # each head pair partition-half stays independent
                    nc.gpsimd.memset(kv2lo[HD:P, :].bitcast(F32), 0.0)
                    nc.gpsimd.memset(kv2hi[0:HD, :].bitcast(F32), 0.0)
                    # v1 cols 65:128 zero -> psum rows 65:128 stay finite zeros
                    nc.gpsimd.memset(v1_sb[:, :, HD + 1:P], 0.0)

                    def rope(x, tabsl, rows):
                        # x: [128, TCH] slice, roped in place on rows [0:rows].
                        # rot_half via PE permutation matmul (psum), then only
                        # partition-aligned TTs: rot*=sin; x*=cos; x+=rot.
                        rot = psrot.tile([P, TCH], F32, tag="rot")
                        nc.tensor.matmul(
                            rot[:], lhsT=prot_sb[:], rhs=x, start=True, stop=True)
                        c = ck_sb[:, tabsl]
                        s = sk_sb[:, tabsl]
                        xr = x[0:rows, :] if rows < P else x
                        nc.vector.tensor_mul(
                            out=rot[0:rows, :], in0=rot[0:rows, :],
                            in1=s[0:rows, :])
                        nc.vector.tensor_mul(out=xr, in0=xr, in1=c[0:rows, :])
                        nc.vector.tensor_add(out=xr, in0=xr, in1=rot[0:rows, :])

                    vstgs = []
                    for tci in range(NTCH):
                        ts = slice(tci * TCH, (tci + 1) * TCH)
                        t0 = (tci * TCH) % S
                        tabsl = slice(t0, t0 + TCH)
                        if tci == 0:
                            nc.sync.dma_start(wkv_sb[:], wkv3)
                            for dq in range(4):
                                nc.sync.dma_start(
                                    wqg_sb[:, dq * 4:(dq + 1) * 4, :],
                                    wqg3[:, dq * 4:(dq + 1) * 4, :])
                        ht = hst_pool.tile([P, DC * TCH], BF16, tag="hst")
                        nc.sync.dma_start(
                            ht[:].rearrange("p (o t) -> p o t", o=DC),
                            hsT3[:, :, ts])
                        if tci == 0:
                            nc.sync.dma_start(ck_sb[:], ck.ap())
                            nc.sync.dma_start(sk_sb[:], sk.ap())
                        pss = [ps512.tile([P, TCH], F32, tag="ps512",
                                          name=f"psA{_m}") for _m in range(5)]
                        for dc in range(DC):
                            for mt in range(5):  # 0: kv, 1-2: q pairs, 3-4: gate
                                if mt == 0:
                                    w = wkv_sb[:, dc, :]
                                else:
                                    w = wqg_sb[:, dc, (mt - 1) * P:mt * P]
                                nc.tensor.matmul(
                                    pss[mt][:],
                                    lhsT=w,
                                    rhs=ht[:, dc * TCH:(dc + 1) * TCH],
                                    start=(dc == 0),
                                    stop=(dc == DC - 1),
                                )
                        # evictions
                        nc.vector.tensor_copy(kv2lo[0:HD, ts], pss[0][0:HD, :])
                        vstg = vst.tile([HD, TCH], F32R, tag="vst")
                        nc.vector.tensor_copy(vstg[:], pss[0][HD:P, :])
                        nc.scalar.copy(qP_sb[:, 0, ts], pss[1][:])
                        nc.scalar.copy(qP_sb[:, 1, ts], pss[2][:])
                        ev = evc.tile([P, 2, TCH], BF16, tag="ev")
                        for mo in range(2):
                            nc.scalar.activation(ev[:, mo, :], pss[3 + mo][:],
                                                 AF.Sigmoid)
                        nc.sync.dma_start(gbuf.ap()[:, :, ts], ev[:])
                        # rope this chunk (overlaps next chunk's matmuls)
                        rope(kv2lo[:, ts], tabsl, HD)
                        nc.vector.tensor_copy(kv2hi[HD:P, ts], kv2lo[0:HD, ts])
                        rope(qP_sb[:, 0, ts], tabsl, P)
                        rope(qP_sb[:, 1, ts], tabsl, P)
                        vstgs.append(vstg)
                        # per-batch v transposes: batch-0's run under batch-1's
                        # projection matmuls
                        if tci in (3, 7):
                            for tcj in range(tci - 3, tci + 1):
                                for j4 in range(TCH // P):
                                    jc = tcj * (TCH // P) + j4
                                    vt_ps = psrot.tile([P, HD], F32R, tag="rot")
                                    nc.tensor.transpose(
                                        vt_ps[:],
                                        vstgs[tcj][:, j4 * P:(j4 + 1) * P],
                                        ident_sb[:],
                                    )
                                    nc.scalar.copy(v1_sb[:, jc, 0:HD], vt_ps[:])
                    nc.sync.dma_start(v1_sb[:, :, HD:HD + 1], onesd.ap()[:, :, None])

                # ---- stage B: attention + deferred normalize + stage C ----
                with (
                    nc.named_scope("stageB"),
                    tc.tile_pool(name="exp", bufs=3) as exp_pool,
                    tc.tile_pool(name="expi", bufs=2) as expi_pool,
                    tc.tile_pool(name="small", bufs=2) as small,
                    tc.tile_pool(name="wop", bufs=1) as wop,
                    tc.tile_pool(name="sgp", bufs=1) as sgp,
                    tc.tile_pool(name="evC", bufs=3) as evC,
                    tc.tile_pool(name="agp", bufs=1) as agp,
                    tc.tile_pool(name="pssc", bufs=2, space="PSUM") as pssc,
                    tc.tile_pool(name="psat", bufs=2, space="PSUM") as psat,
                ):
                    IW = 2 * TCH               # 1024-wide i-half
                    wo_sb = wop.tile([P, 2, D], BF16)
                    nc.sync.dma_start(wo_sb[:], wo3)
                    attnG_sb = agp.tile([P, 2, T], BF16)
                    sgt = [sgp.tile([P, 2, S], BF16, name=f"sgt{b}")
                           for b in range(B)]
                    den = sgp.tile([P, 2, IW], F32R)  # ping-pong by ihalf parity
                    nc.vector.memset(den[0:HD, :, :].bitcast(F32), 0.0)

                    pending = []   # deferred normalize args

                    def emit_C(trange):
                        # C psum comes from the psat ring (flush() must have
                        # drained pending normalizes first)
                        for tt in trange:
                            tsl = slice(tt * P, (tt + 1) * P)
                            ev = evC.tile([P, D], BF16, tag="evC")
                            for oh in range(2):
                                ps = psat.tile([P, IW], F32, tag="psat")
                                for mc in range(2):
                                    for ii in range(2):
                                        o0 = oh * IW + ii * TCH
                                        nc.tensor.matmul(
                                            ps[:, ii * TCH:(ii + 1) * TCH],
                                            lhsT=attnG_sb[:, mc, tsl],
                                            rhs=wo_sb[:, mc, o0:o0 + TCH],
                                            start=(mc == 0),
                                            stop=(mc == 1),
                                        )
                                osl = slice(oh * IW, (oh + 1) * IW)
                                if oh == 0:
                                    nc.vector.tensor_copy(ev[:, osl], ps[:])
                                else:
                                    nc.scalar.copy(ev[:, osl], ps[:])
                            nc.sync.dma_start(out.ap()[tsl, :], ev[:])

                    def normalize(b, h, ih, a_ps):
                        # denom is psum row 64 (ones-col accumulation); rows
                        # 65:128 are zeros. Copy rows 64:128 aligned into den
                        # (rows 0:64 pre-zeroed once), broadcast row 64 to all
                        # partitions via the row-64 selector matmul, 1/x, then
                        # the gated muls (gate mul on the Pool engine).
                        hp = (h % 2) * HD
                        mo = h // 2
                        par = ih % 2
                        sl = slice(ih * IW, (ih + 1) * IW)
                        osl = slice(b * S + ih * IW, b * S + (ih + 1) * IW)
                        nc.vector.tensor_copy(den[HD:P, par, :], a_ps[HD:P, :])
                        bc_ps = pssc.tile([P, IW], F32, tag="pssc")
                        for ii in range(2):
                            nc.tensor.matmul(
                                bc_ps[:, ii * TCH:(ii + 1) * TCH],
                                lhsT=osel_sb[:],
                                rhs=den[:, par, ii * TCH:(ii + 1) * TCH],
                                start=True, stop=True,
                            )
                        rcp = small.tile([P, IW], F32, tag="rcp")
                        # reciprocal_approx_fast misbehaves at base partition
                        # 64; bc rows are all the denominator, so always use
                        # rows 0:64 (mixed-space mul allows base mismatch)
                        nc.vector.reciprocal_approx_fast(
                            out=rcp[0:HD, :], in_=bc_ps[0:HD, :])
                        ag = attnG_sb[hp:hp + HD, mo, osl]
                        nc.vector.tensor_mul(
                            out=ag, in0=a_ps[0:HD, :], in1=rcp[0:HD, :])
                        nc.gpsimd.tensor_mul(
                            out=ag, in0=ag, in1=sgt[b][hp:hp + HD, mo, sl])

                    def flush():
                        while pending:
                            normalize(*pending.pop(0))

                    for b in range(B):
                        nc.sync.dma_start(
                            sgt[b][:], gbuf.ap()[:, :, b * S:(b + 1) * S])
                        for h in range(NH):
                            mo = h // 2
                            kvt = kv2lo if h % 2 == 0 else kv2hi
                            for ih in range(2):
                                i0 = b * S + ih * IW
                                isl = slice(i0, i0 + IW)
                                a_ps = psat.tile([P, IW], F32, tag="psat")

                                def scores_exp(jc):
                                    jsl = slice(b * S + jc * P,
                                                b * S + (jc + 1) * P)
                                    s_ps = pssc.tile([P, IW], F32, tag="pssc")
                                    for ii in range(2):
                                        nc.tensor.matmul(
                                            s_ps[:, ii * TCH:(ii + 1) * TCH],
                                            lhsT=kvt[:, jsl],
                                            rhs=qP_sb[:, mo,
                                                      i0 + ii * TCH:
                                                      i0 + (ii + 1) * TCH],
                                            start=True,
                                            stop=True,
                                        )
                                    if jc in DVE_JCS:
                                        yi = expi_pool.tile([P, IW], I32,
                                                            tag="expi")
                                        nc.vector.tensor_scalar(
                                            out=yi[:], in0=s_ps[:],
                                            scalar1=AS_CONST, scalar2=BS_CONST,
                                            op0=ALU.mult, op1=ALU.add,
                                        )
                                        return yi
                                    ex = exp_pool.tile([P, IW], BF16,
                                                       tag="exp")
                                    nc.scalar.activation(
                                        ex[:], s_ps[:], AF.Exp, scale=0.125)
                                    return ex

                                def attn_acc(jc, ex):
                                    if ex.dtype == I32:
                                        # top 16 bits of the Schraudolph int32
                                        # ARE the bf16 exp value; emit per
                                        # psum-bank halves so the strided AP
                                        # survives codegen
                                        for ii in range(2):
                                            r = ex[:, ii * TCH:(ii + 1) * TCH]
                                            r = r.bitcast(BF16).rearrange(
                                                "p (n t) -> p n t", t=2)[:, :, 1]
                                            nc.tensor.matmul(
                                                a_ps[:, ii * TCH:(ii + 1) * TCH],
                                                lhsT=v1_sb[:, b * SJ + jc, :],
                                                rhs=r,
                                                start=(jc == 0),
                                                stop=(jc == SJ - 1),
                                            )
                                        return
                                    for ii in range(2):
                                        nc.tensor.matmul(
                                            a_ps[:, ii * TCH:(ii + 1) * TCH],
                                            lhsT=v1_sb[:, b * SJ + jc, :],
                                            rhs=ex[:, ii * TCH:(ii + 1) * TCH],
                                            start=(jc == 0),
                                            stop=(jc == SJ - 1),
                                        )

                                prev = scores_exp(0)
                                for jc in range(1, SJ):
                                    cur = scores_exp(jc)
                                    if jc == 1:
                                        flush()
                                    attn_acc(jc - 1, prev)
                                    prev = cur
                                attn_acc(SJ - 1, prev)
                                pending.append((b, h, ih, a_ps))
                            # interleave prev batch's out-projection chunks;
                            # flush first so no pending normalize still needs
                            # a psat slot the C tiles will recycle
                            if b == 1:
                                flush()
                                emit_C(range(h * 4, (h + 1) * 4))
                    flush()
                    emit_C(range(NT // 2, NT))

    nc.compile()
    return nc


_NC_CACHE = None


def _get_nc(nreps=1):
    global _NC_CACHE
    if _NC_CACHE is None:
        _NC_CACHE = {}
    if nreps not in _NC_CACHE:
        _NC_CACHE[nreps] = build_nc(nreps)
    return _NC_CACHE[nreps]


def _dup_rows(tab64):
    """[64, S] -> [128, S] with both partition halves holding the table."""
    return np.ascontiguousarray(np.concatenate([tab64, tab64], axis=0))


def _prep_inputs(hidden_states, cos, sin, Wq, Wk, Wv, Wo):
    hs = np.asarray(hidden_states, dtype=np.float32)
    cos = np.asarray(cos, dtype=np.float32)
    sin = np.asarray(sin, dtype=np.float32)
    Wq = np.asarray(Wq, dtype=np.float32)
    Wk = np.asarray(Wk, dtype=np.float32)
    Wv = np.asarray(Wv, dtype=np.float32)
    Wo = np.asarray(Wo, dtype=np.float32)

    bf16 = mybir.dt.np(BF16)
    hsT = np.ascontiguousarray(hs.reshape(T, D).T).astype(bf16)

    cosT = cos.T                                     # [64, S]
    sinT = sin.T
    sin_signed = np.concatenate([-sinT[:HH], sinT[HH:]], axis=0)
    osel = np.zeros((P, P), np.float32)
    osel[HD, :] = 1.0
    prot = np.zeros((P, P), np.float32)
    for k in range(P):
        prot[k, k ^ HH] = 1.0
    common = {
        "hsT": hsT,
        "ck": _dup_rows(cosT),
        "sk": _dup_rows(sin_signed),
        "ident": np.eye(HD, dtype=np.float32),
        "osel": osel,
        "prot": prot,
        "ones": np.ones((P, B * SJ), mybir.dt.np(BF16)),
    }
    in_maps = []
    for c in range(NCORES):
        qcols = Wq[:, c * MQ:(c + 1) * MQ]
        gcols = Wq[:, H * HD + c * MQ: H * HD + (c + 1) * MQ]
        in_maps.append(
            {
                **common,
                "wqg": np.ascontiguousarray(
                    np.concatenate([qcols, gcols], axis=1)
                ).astype(bf16),
                "wkv": np.ascontiguousarray(
                    np.concatenate(
                        [Wk[:, c * HD:(c + 1) * HD], Wv[:, c * HD:(c + 1) * HD]],
                        axis=1,
                    )
                ).astype(bf16),
                "wo": np.ascontiguousarray(Wo[c * MQ:(c + 1) * MQ, :]).astype(bf16),
            }
        )
    return in_maps


def kernel(hidden_states, cos, sin, Wq, Wk, Wv, Wo, _trace=False, _trace_kwargs=None):
    nc = _get_nc()
    in_maps = _prep_inputs(hidden_states, cos, sin, Wq, Wk, Wv, Wo)
    res = run_bass_kernel_spmd(
        nc, in_maps, list(range(NCORES)), trace=_trace, **(_trace_kwargs or {})
    )
    total = res.results[0]["out"].astype(np.float32).copy()
    for c in range(1, NCORES):
        total += res.results[c]["out"]
    out = total.reshape(B, S, D)
    if _trace:
        kernel._last_results = res
    return out


# revision 21
# speedup vs baseline: 1.1656x; 1.1656x over previous
"""Tensor-parallel GQA attention (sigmoid-gated) for Trainium2, 8 NeuronCores.

Problem: B=2, S=2048, D=2048, H=32 q-heads, KV=8 kv-heads, HD=64 (GQA groups=4),
RoPE on q/k, full (non-causal) softmax, sigmoid(gate) output gating, out proj.

Sharding (tensor-parallel over heads): core c owns q-heads 4c..4c+3, kv-head c,
the matching 256 q-cols + 256 gate-cols of Wq, 64-col slices of Wk/Wv, and rows
256c:256c+256 of Wo. Each core computes a full [B*S, D] partial of the output
projection; the host sums the 8 partials.

Per-core pipeline (matmuls in float32r = full-rate reduced-precision fp32):
  A) projections psum[m,t] += W[d,m].T @ hsT[d,t] with per-dc weight DMAs so
     the first matmul starts as soon as one 128-row weight slice lands.
     q lands in head-PAIR layout qP[128, 2, T] (head 2m at partitions 0:64,
     head 2m+1 at 64:128); k is evicted to kv2lo (rows 0:64, zeros above) and
     mirrored into kv2hi (rows 64:128, zeros below) so each head's scores
     contract K=128 against a zero-padded k copy matching its partition half.
     RoPE is applied per 512-column chunk right after eviction (DVE + a few
     ops on the idle Pool/GPSIMD engine), fully overlapped with the next
     chunk's matmuls.  Gate columns get sigmoid at eviction and round-trip
     through DRAM (SBUF pressure).
  B) attention in scoresT orientation per 512-col i-segment, j in pairs:
     scoresT[j,i] = kv[hd,j].T @ qP[hd,i] for two j-tiles into one
     [128,1024] psum; probs via exp(s/8) on ACT (scale=0.125) for most
     pairs and a 1-op Schraudolph bit-trick exp on DVE for a few pairs
     (splits the activation load; softmax renormalization absorbs most of
     the approximation error).  attnT[hd,i] += v1[j, hd|1].T @ probs --
     v1's ones-column accumulates denominators in psum row 64 for free.
     Normalization is deferred one segment (runs under the next segment's
     matmuls): denom row -> SBUF, broadcast across partitions with a
     row-64-selector matmul, reciprocal_approx_fast, then gated muls
     (gate mul on the Pool engine).
  C) out[t,dout] += attnG[m,t].T @ Wo[m,dout] partials, interleaved per
     4-t-tile chunks between batch-1 heads so psum/eviction load spreads.
"""

import sys

sys.path.insert(0, "/opt/trn_rl_repo")

import numpy as np

import concourse.bass as bass  # noqa: F401
import concourse.mybir as mybir
import concourse.tile as tile
from concourse import bacc
from concourse.bass_utils import run_bass_kernel_spmd

F32 = mybir.dt.float32
F32R = mybir.dt.float32r
I32 = mybir.dt.int32
BF16 = mybir.dt.bfloat16
AF = mybir.ActivationFunctionType
ALU = mybir.AluOpType

P = 128
B, S, D = 2, 2048, 2048
T = B * S                  # 4096 token rows (batch folded)
H, KV, HD = 32, 8, 64
HH = HD // 2               # 32
NCORES = 8
NH = H // NCORES           # 4 q-heads per core
MQ = NH * HD               # 256 q-cols per core
DC = D // P                # 16 contraction chunks
TCH = 512                  # moving-dim chunk
NTCH = T // TCH            # 8
SJ = S // P                # 16 key chunks per batch
NSEG = S // TCH            # 4 i-segments per batch
NT = T // P                # 32 t-tiles
NP = SJ // 2               # 8 j-pairs per segment

# Schraudolph fast-exp (DVE): exp(s/8) ~= bitcast_i32_f32(s*AS + BS)
AS_CONST = float((1 << 23) * 1.4426950408889634 * 0.125)
BS_CONST = float(127 * (1 << 23) - 449200)
# which j-chunks (of 16 per i-half) compute probs on DVE instead of ACT
DVE_JCS = (3, 7, 11)


def build_nc(nreps=1):
    nc = bacc.Bacc("TRN2", target_bir_lowering=False, debug=False)

    hsT = nc.dram_tensor("hsT", [D, T], BF16, kind="ExternalInput")
    wqg = nc.dram_tensor("wqg", [D, 2 * MQ], BF16, kind="ExternalInput")
    wkv = nc.dram_tensor("wkv", [D, 2 * HD], BF16, kind="ExternalInput")
    wo = nc.dram_tensor("wo", [MQ, D], BF16, kind="ExternalInput")
    # rope tables, [128, S] with rows duplicated (row p holds entry p % 64)
    ck = nc.dram_tensor("ck", [P, S], F32, kind="ExternalInput")   # cos
    sk = nc.dram_tensor("sk", [P, S], F32, kind="ExternalInput")   # signed sin
    identd = nc.dram_tensor("ident", [HD, HD], F32R, kind="ExternalInput")
    oseld = nc.dram_tensor("osel", [P, P], F32R, kind="ExternalInput")  # row64=1
    protd = nc.dram_tensor("prot", [P, P], F32R, kind="ExternalInput")  # xor-32 perm
    onesd = nc.dram_tensor("ones", [P, B * SJ], BF16, kind="ExternalInput")
    out = nc.dram_tensor("out", [T, D], BF16, kind="ExternalOutput")

    gbuf = nc.dram_tensor("gbuf", [P, 2, T], BF16)  # sigmoid(gate), internal

    hsT3 = hsT.ap().rearrange("(o p) t -> p o t", p=P)   # [128, 16, 4096]
    wqg3 = wqg.ap().rearrange("(o p) m -> p o m", p=P)   # [128, 16, 512]
    wkv3 = wkv.ap().rearrange("(o p) m -> p o m", p=P)   # [128, 16, 128]
    wo3 = wo.ap().rearrange("(o p) n -> p o n", p=P)     # [128, 2, 2048]

    with tile.TileContext(nc) as tc:
        for _rep in range(nreps):
            with (
                tc.tile_pool(name="const", bufs=1) as const,
                tc.tile_pool(name="big", bufs=1) as big,
            ):
                ident_sb = const.tile([HD, HD], F32R)
                osel_sb = const.tile([P, P], F32R)  # row 64 ones, rest zero
                prot_sb = const.tile([P, P], F32R)  # xor-32 permutation
                nc.sync.dma_start(ident_sb[:], identd.ap())
                nc.sync.dma_start(osel_sb[:], oseld.ap())
                nc.sync.dma_start(prot_sb[:], protd.ap())

                # ---- persistent activations ----
                qP_sb = big.tile([P, 2, T], F32R)    # head pairs, roped in place
                kv2lo = big.tile([P, T], F32R)       # roped k rows 0:64, 0 above
                kv2hi = big.tile([P, T], F32R)       # 0 below, roped k rows 64:128
                v1_sb = big.tile([P, B * SJ, P], BF16)  # v | ones col | zeros

                # ---- stage A: projections + fused rope ----
                with (
                    nc.named_scope("stageA"),
                    tc.tile_pool(name="wpool", bufs=1) as wpool,
                    tc.tile_pool(name="tab", bufs=1) as tab,
                    tc.tile_pool(name="hst", bufs=3) as hst_pool,
                    tc.tile_pool(name="vst", bufs=8) as vst,
                    tc.tile_pool(name="evc", bufs=3) as evc,
                    tc.tile_pool(name="ps512", bufs=5, space="PSUM") as ps512,
                    tc.tile_pool(name="psrot", bufs=2, space="PSUM") as psrot,
                ):
                    ck_sb = tab.tile([P, S], F32)
                    sk_sb = tab.tile([P, S], F32)

                    wqg_sb = wpool.tile([P, DC, 2 * MQ], BF16)
                    wkv_sb = wpool.tile([P, DC, 2 * HD], BF16)
                    # zero pads: scores contract K=128 with a zeroed half so
                    # each head pair partition-half stays independent
                    nc.gpsimd.memset(kv2lo[HD:P, :].bitcast(F32), 0.0)
                    nc.gpsimd.memset(kv2hi[0:HD, :].bitcast(F32), 0.0)
                    # v1 cols 65:128 zero -> psum rows 65:128 stay finite zeros
                    nc.gpsimd.memset(v1_sb[:, :, HD + 1:P], 0.0)

                    def rope(x, tabsl, rows):
                        # x: [128, TCH] slice, roped in place on rows [0:rows].
                        # rot_half via PE permutation matmul (psum), then only
                        # partition-aligned TTs: rot*=sin; x*=cos; x+=rot.
                        rot = psrot.tile([P, TCH], F32, tag="rot")
                        nc.tensor.matmul(
                            rot[:], lhsT=prot_sb[:], rhs=x, start=True, stop=True)
                        c = ck_sb[:, tabsl]
                        s = sk_sb[:, tabsl]
                        xr = x[0:rows, :] if rows < P else x
                        nc.vector.tensor_mul(
                            out=rot[0:rows, :], in0=rot[0:rows, :],
                            in1=s[0:rows, :])
                        nc.vector.tensor_mul(out=xr, in0=xr, in1=c[0:rows, :])
                        nc.vector.tensor_add(out=xr, in0=xr, in1=rot[0:rows, :])

                    vstgs = []
                    for tci in range(NTCH):
                        ts = slice(tci * TCH, (tci + 1) * TCH)
                        t0 = (tci * TCH) % S
                        tabsl = slice(t0, t0 + TCH)
                        if tci == 0:
                            nc.sync.dma_start(wkv_sb[:], wkv3)
                            for dq in range(4):
                                nc.sync.dma_start(
                                    wqg_sb[:, dq * 4:(dq + 1) * 4, :],
                                    wqg3[:, dq * 4:(dq + 1) * 4, :])
                        ht = hst_pool.tile([P, DC * TCH], BF16, tag="hst")
                        nc.sync.dma_start(
                            ht[:].rearrange("p (o t) -> p o t", o=DC),
                            hsT3[:, :, ts])
                        if tci == 0:
                            nc.sync.dma_start(ck_sb[:], ck.ap())
                            nc.sync.dma_start(sk_sb[:], sk.ap())
                        pss = [ps512.tile([P, TCH], F32, tag="ps512",
                                          name=f"psA{_m}") for _m in range(5)]
                        for dc in range(DC):
                            for mt in range(5):  # 0: kv, 1-2: q pairs, 3-4: gate
                                if mt == 0:
                                    w = wkv_sb[:, dc, :]
                                else:
                                    w = wqg_sb[:, dc, (mt - 1) * P:mt * P]
                                nc.tensor.matmul(
                                    pss[mt][:],
                                    lhsT=w,
                                    rhs=ht[:, dc * TCH:(dc + 1) * TCH],
                                    start=(dc == 0),
                                    stop=(dc == DC - 1),
                                )
                        # evictions
                        nc.vector.tensor_copy(kv2lo[0:HD, ts], pss[0][0:HD, :])
                        vstg = vst.tile([HD, TCH], F32R, tag="vst")
                        nc.vector.tensor_copy(vstg[:], pss[0][HD:P, :])
                        nc.scalar.copy(qP_sb[:, 0, ts], pss[1][:])
                        nc.scalar.copy(qP_sb[:, 1, ts], pss[2][:])
                        ev = evc.tile([P, 2, TCH], BF16, tag="ev")
                        for mo in range(2):
                            nc.scalar.activation(ev[:, mo, :], pss[3 + mo][:],
                                                 AF.Sigmoid)
                        nc.sync.dma_start(gbuf.ap()[:, :, ts], ev[:])
                        # rope this chunk (overlaps next chunk's matmuls)
                        rope(kv2lo[:, ts], tabsl, HD)
                        nc.vector.tensor_copy(kv2hi[HD:P, ts], kv2lo[0:HD, ts])
                        rope(qP_sb[:, 0, ts], tabsl, P)
                        rope(qP_sb[:, 1, ts], tabsl, P)
                        vstgs.append(vstg)
                    # batched v transposes: PE transpose-mode entered once
                    for tci in range(NTCH):
                        for j4 in range(TCH // P):
                            jc = tci * (TCH // P) + j4
                            vt_ps = psrot.tile([P, HD], F32R, tag="rot")
                            nc.tensor.transpose(
                                vt_ps[:],
                                vstgs[tci][:, j4 * P:(j4 + 1) * P],
                                ident_sb[:],
                            )
                            nc.scalar.copy(v1_sb[:, jc, 0:HD], vt_ps[:])
                    nc.sync.dma_start(v1_sb[:, :, HD:HD + 1], onesd.ap()[:, :, None])

                # ---- stage B: attention + deferred normalize + stage C ----
                with (
                    nc.named_scope("stageB"),
                    tc.tile_pool(name="exp", bufs=3) as exp_pool,
                    tc.tile_pool(name="expi", bufs=2) as expi_pool,
                    tc.tile_pool(name="small", bufs=2) as small,
                    tc.tile_pool(name="wop", bufs=1) as wop,
                    tc.tile_pool(name="sgp", bufs=1) as sgp,
                    tc.tile_pool(name="evC", bufs=3) as evC,
                    tc.tile_pool(name="agp", bufs=1) as agp,
                    tc.tile_pool(name="pssc", bufs=2, space="PSUM") as pssc,
                    tc.tile_pool(name="psat", bufs=2, space="PSUM") as psat,
                ):
                    IW = 2 * TCH               # 1024-wide i-half
                    wo_sb = wop.tile([P, 2, D], BF16)
                    nc.sync.dma_start(wo_sb[:], wo3)
                    attnG_sb = agp.tile([P, 2, T], BF16)
                    sgt = [sgp.tile([P, 2, S], BF16, name=f"sgt{b}")
                           for b in range(B)]
                    den = sgp.tile([P, 2, IW], F32R)  # ping-pong by ihalf parity
                    nc.vector.memset(den[0:HD, :, :].bitcast(F32), 0.0)

                    pending = []   # deferred normalize args

                    def emit_C(trange):
                        # C psum comes from the psat ring (flush() must have
                        # drained pending normalizes first)
                        for tt in trange:
                            tsl = slice(tt * P, (tt + 1) * P)
                            ev = evC.tile([P, D], BF16, tag="evC")
                            for oh in range(2):
                                ps = psat.tile([P, IW], F32, tag="psat")
                                for mc in range(2):
                                    for ii in range(2):
                                        o0 = oh * IW + ii * TCH
                                        nc.tensor.matmul(
                                            ps[:, ii * TCH:(ii + 1) * TCH],
                                            lhsT=attnG_sb[:, mc, tsl],
                                            rhs=wo_sb[:, mc, o0:o0 + TCH],
                                            start=(mc == 0),
                                            stop=(mc == 1),
                                        )
                                osl = slice(oh * IW, (oh + 1) * IW)
                                if oh == 0:
                                    nc.vector.tensor_copy(ev[:, osl], ps[:])
                                else:
                                    nc.scalar.copy(ev[:, osl], ps[:])
                            nc.sync.dma_start(out.ap()[tsl, :], ev[:])

                    def normalize(b, h, ih, a_ps):
                        # denom is psum row 64 (ones-col accumulation); rows
                        # 65:128 are zeros. Copy rows 64:128 aligned into den
                        # (rows 0:64 pre-zeroed once), broadcast row 64 to all
                        # partitions via the row-64 selector matmul, 1/x, then
                        # the gated muls (gate mul on the Pool engine).
                        hp = (h % 2) * HD
                        mo = h // 2
                        par = ih % 2
                        sl = slice(ih * IW, (ih + 1) * IW)
                        osl = slice(b * S + ih * IW, b * S + (ih + 1) * IW)
                        nc.vector.tensor_copy(den[HD:P, par, :], a_ps[HD:P, :])
                        bc_ps = pssc.tile([P, IW], F32, tag="pssc")
                        for ii in range(2):
                            nc.tensor.matmul(
                                bc_ps[:, ii * TCH:(ii + 1) * TCH],
                                lhsT=osel_sb[:],
                                rhs=den[:, par, ii * TCH:(ii + 1) * TCH],
                                start=True, stop=True,
                            )
                        rcp = small.tile([P, IW], F32, tag="rcp")
                        # reciprocal_approx_fast misbehaves at base partition
                        # 64; bc rows are all the denominator, so always use
                        # rows 0:64 (mixed-space mul allows base mismatch)
                        nc.vector.reciprocal_approx_fast(
                            out=rcp[0:HD, :], in_=bc_ps[0:HD, :])
                        ag = attnG_sb[hp:hp + HD, mo, osl]
                        nc.vector.tensor_mul(
                            out=ag, in0=a_ps[0:HD, :], in1=rcp[0:HD, :])
                        nc.gpsimd.tensor_mul(
                            out=ag, in0=ag, in1=sgt[b][hp:hp + HD, mo, sl])

                    def flush():
                        while pending:
                            normalize(*pending.pop(0))

                    for b in range(B):
                        nc.sync.dma_start(
                            sgt[b][:], gbuf.ap()[:, :, b * S:(b + 1) * S])
                        for h in range(NH):
                            mo = h // 2
                            kvt = kv2lo if h % 2 == 0 else kv2hi
                            for ih in range(2):
                                i0 = b * S + ih * IW
                                isl = slice(i0, i0 + IW)
                                a_ps = psat.tile([P, IW], F32, tag="psat")

                                def scores_exp(jc):
                                    jsl = slice(b * S + jc * P,
                                                b * S + (jc + 1) * P)
                                    s_ps = pssc.tile([P, IW], F32, tag="pssc")
                                    for ii in range(2):
                                        nc.tensor.matmul(
                                            s_ps[:, ii * TCH:(ii + 1) * TCH],
                                            lhsT=kvt[:, jsl],
                                            rhs=qP_sb[:, mo,
                                                      i0 + ii * TCH:
                                                      i0 + (ii + 1) * TCH],
                                            start=True,
                                            stop=True,
                                        )
                                    if jc in DVE_JCS:
                                        yi = expi_pool.tile([P, IW], I32,
                                                            tag="expi")
                                        nc.vector.tensor_scalar(
                                            out=yi[:], in0=s_ps[:],
                                            scalar1=AS_CONST, scalar2=BS_CONST,
                                            op0=ALU.mult, op1=ALU.add,
                                        )
                                        return yi
                                    ex = exp_pool.tile([P, IW], BF16,
                                                       tag="exp")
                                    nc.scalar.activation(
                                        ex[:], s_ps[:], AF.Exp, scale=0.125)
                                    return ex

                                def attn_acc(jc, ex):
                                    if ex.dtype == I32:
                                        # top 16 bits of the Schraudolph int32
                                        # ARE the bf16 exp value; emit per
                                        # psum-bank halves so the strided AP
                                        # survives codegen
                                        for ii in range(2):
                                            r = ex[:, ii * TCH:(ii + 1) * TCH]
                                            r = r.bitcast(BF16).rearrange(
                                                "p (n t) -> p n t", t=2)[:, :, 1]
                                            nc.tensor.matmul(
                                                a_ps[:, ii * TCH:(ii + 1) * TCH],
                                                lhsT=v1_sb[:, b * SJ + jc, :],
                                                rhs=r,
                                                start=(jc == 0),
                                                stop=(jc == SJ - 1),
                                            )
                                        return
                                    for ii in range(2):
                                        nc.tensor.matmul(
                                            a_ps[:, ii * TCH:(ii + 1) * TCH],
                                            lhsT=v1_sb[:, b * SJ + jc, :],
                                            rhs=ex[:, ii * TCH:(ii + 1) * TCH],
                                            start=(jc == 0),
                                            stop=(jc == SJ - 1),
                                        )

                                prev = scores_exp(0)
                                for jc in range(1, SJ):
                                    cur = scores_exp(jc)
                                    if jc == 1:
                                        flush()
                                    attn_acc(jc - 1, prev)
                                    prev = cur
                                attn_acc(SJ - 1, prev)
                                pending.append((b, h, ih, a_ps))
                            # interleave prev batch's out-projection chunks;
                            # flush first so no pending normalize still needs
                            # a psat slot the C tiles will recycle
                            if b == 1:
                                flush()
                                emit_C(range(h * 4, (h + 1) * 4))
                    flush()
                    emit_C(range(NT // 2, NT))

    nc.compile()
    return nc


_NC_CACHE = None


def _get_nc(nreps=1):
    global _NC_CACHE
    if _NC_CACHE is None:
        _NC_CACHE = {}
    if nreps not in _NC_CACHE:
        _NC_CACHE[nreps] = build_nc(nreps)
    return _NC_CACHE[nreps]


def _dup_rows(tab64):
    """[64, S] -> [128, S] with both partition halves holding the table."""
    return np.ascontiguousarray(np.concatenate([tab64, tab64], axis=0))


def _prep_inputs(hidden_states, cos, sin, Wq, Wk, Wv, Wo):
    hs = np.asarray(hidden_states, dtype=np.float32)
    cos = np.asarray(cos, dtype=np.float32)
    sin = np.asarray(sin, dtype=np.float32)
    Wq = np.asarray(Wq, dtype=np.float32)
    Wk = np.asarray(Wk, dtype=np.float32)
    Wv = np.asarray(Wv, dtype=np.float32)
    Wo = np.asarray(Wo, dtype=np.float32)

    bf16 = mybir.dt.np(BF16)
    hsT = np.ascontiguousarray(hs.reshape(T, D).T).astype(bf16)

    cosT = cos.T                                     # [64, S]
    sinT = sin.T
    sin_signed = np.concatenate([-sinT[:HH], sinT[HH:]], axis=0)
    osel = np.zeros((P, P), np.float32)
    osel[HD, :] = 1.0
    prot = np.zeros((P, P), np.float32)
    for k in range(P):
        prot[k, k ^ HH] = 1.0
    common = {
        "hsT": hsT,
        "ck": _dup_rows(cosT),
        "sk": _dup_rows(sin_signed),
        "ident": np.eye(HD, dtype=np.float32),
        "osel": osel,
        "prot": prot,
        "ones": np.ones((P, B * SJ), mybir.dt.np(BF16)),
    }
    in_maps = []
    for c in range(NCORES):
        qcols = Wq[:, c * MQ:(c + 1) * MQ]
        gcols = Wq[:, H * HD + c * MQ: H * HD + (c + 1) * MQ]
        in_maps.append(
            {
                **common,
                "wqg": np.ascontiguousarray(
                    np.concatenate([qcols, gcols], axis=1)
                ).astype(bf16),
                "wkv": np.ascontiguousarray(
                    np.concatenate(
                        [Wk[:, c * HD:(c + 1) * HD], Wv[:, c * HD:(c + 1) * HD]],
                        axis=1,
                    )
                ).astype(bf16),
                "wo": np.ascontiguousarray(Wo[c * MQ:(c + 1) * MQ, :]).astype(bf16),
            }
        )
    return in_maps


def kernel(hidden_states, cos, sin, Wq, Wk, Wv, Wo, _trace=False, _trace_kwargs=None):
    nc = _get_nc()
    in_maps = _prep_inputs(hidden_states, cos, sin, Wq, Wk, Wv, Wo)
    res = run_bass_kernel_spmd(
        nc, in_maps, list(range(NCORES)), trace=_trace, **(_trace_kwargs or {})
    )
    total = res.results[0]["out"].astype(np.float32).copy()
    for c in range(1, NCORES):
        total += res.results[c]["out"]
    out = total.reshape(B, S, D)
    if _trace:
        kernel._last_results = res
    return out
